# revision 42
# baseline (speedup 1.0000x reference)
"""Trainium2 Bass kernel for nn_CABlock_26912265077025.

Architecture: CA-gating block (pools -> conv -> sigmoid gates -> x*gd*gh*gw)
followed by a 12000->4096->512->3 MLP and row L2-normalization.

Strategy: pure data parallelism over the batch across 8 NeuronCores
(512 rows each). The dominant 12032x4096 GEMM (and the 4096x512 GEMM) run
as fp8e4 DoubleRow matmuls (2 k-tiles / 0.5 cycles per row on the PE) using
a 3-term residual decomposition that keeps bf16-level accuracy:
  z = zh + zl, w = wh + wl (all fp8, residuals at the same scale)
  y ~= zh@wh + (zl@wh + zh@wl)   [the zl@wl term is ~0.1% and dropped]
The two cross terms are computed in a single DoubleRow pass that pairs the
(zl, zh) planes with (wh, wl) planes along the contraction dim, so mm1
costs 0.75x its bf16 PE time at the 4x fp8 rate.

The gated activations are quantized on the fly, packed IN PLACE over the
dead bf16 x bytes per column-quarter (zl/zh planes of 128 bytes each), via
a 4-pass chain: exp (ACT) -> gate-mul (DVE) -> fp8 copy (DVE/ACT alternate)
-> subtract with direct fp8 output (DVE). Gate selection matmuls also run
as fp8 DoubleRow using exact 0/1 selection weights and a mean-shifted
softplus pair (Lp' = softplus(-T) - ln2, hi+lo fp8), with the e^{-3ln2}
constant folded into the gate exp bias.
"""

from contextlib import ExitStack

import ml_dtypes
import numpy as np

import concourse.bass as bass
import concourse.mybir as mybir
import concourse.tile as tile
from concourse import bacc
from concourse.bass_utils import run_bass_kernel_spmd

N_CORES = 8
B_TOT = 4096
BS = B_TOT // N_CORES            # 512 batch rows per core
F = 12000                        # 3*10*10*40 flattened features
NK = 94                          # ceil(F/128)
FP = NK * 128                    # 12032 (rows F..FP-1 zero-padded)
H1, H2 = 4096, 512
NM1 = H1 // 128                  # 32 mm1 output tiles
NK2, NM2 = H1 // 128, H2 // 128  # 32, 4
KG = 6                           # w1 k-chunks per DMA group in phase D
SELG = 4                         # sel/conversion chunk-group size
NSG = (NK + SELG - 1) // SELG    # 24 groups (last has 2)
WPG = 8                          # wpool chunks per group
NWPG = (NK + WPG - 1) // WPG
P0G = 8                          # pair-0 w1 prefetch chunk-group size
NQ = 4                           # column quarters
QC = BS // NQ                    # 128 cols per quarter

S_Z = 4.0                        # z scale (z4 = 4*x*G)
S_W1 = 1024.0
S1 = S_Z * S_W1                  # 4096
S_Z1 = 4.0
S_W2 = 512.0
S2 = S_Z1 * S_W2                 # 2048
NEG_LN2 = -0.6931471805599453

f32 = mybir.dt.float32
f32r = mybir.dt.float32r
bf16 = mybir.dt.bfloat16
fp8 = mybir.dt.float8e4
np_bf16 = ml_dtypes.bfloat16
np_fp8 = ml_dtypes.float8_e4m3
AF = mybir.ActivationFunctionType
DR = mybir.MatmulPerfMode.DoubleRow

_NC_CACHE = {}

# z tile chunk grouping (all even so chunk-pairs never straddle tiles)
ZGROUPS = [2, 2] + [4] * 22 + [2]
assert sum(ZGROUPS) == NK


def build_nc(W1BUFS=4, SELBUFS=3, GTBUFS=3, STBUFS=3):
    nc = bacc.Bacc(None, target_bir_lowering=False)

    xt_d = nc.dram_tensor("xt", [128, NK, BS], bf16, kind="ExternalInput")
    w1q_d = nc.dram_tensor("w1q", [NM1 // 2, 128, NK, 2, 256], fp8,
                           kind="ExternalInput")
    wpool_d = nc.dram_tensor("wpool", [NWPG, 128, WPG, 50], bf16,
                             kind="ExternalInput")
    rmat_d = nc.dram_tensor("rmat", [50, 180], bf16, kind="ExternalInput")
    ssel_d = nc.dram_tensor("ssel", [NSG, 128, SELG, 2, 128], fp8,
                            kind="ExternalInput")
    w2k_d = nc.dram_tensor("w2k", [NK2 // 2, 128, 2, 2, NM2, 128], fp8,
                           kind="ExternalInput")
    w3h_d = nc.dram_tensor("w3h", [128, NM2, 3], f32r, kind="ExternalInput")
    b1_d = nc.dram_tensor("b1g4", [128, NM1], f32, kind="ExternalInput")
    b2_d = nc.dram_tensor("b2g", [128, NM2], f32, kind="ExternalInput")
    b3_d = nc.dram_tensor("b3r", [1, 3], f32r, kind="ExternalInput")
    idm_d = nc.dram_tensor("idm", [128, 128], f32, kind="ExternalInput")
    out_d = nc.dram_tensor("out", [3, BS], f32, kind="ExternalOutput")

    with tile.TileContext(nc) as tc, ExitStack() as ctx:
        consts = ctx.enter_context(tc.tile_pool(name="consts", bufs=1))

        b1_sb = consts.tile([128, NM1], f32)
        b2_sb = consts.tile([128, NM2], f32)
        b3_sb = consts.tile([1, 3], f32r)
        w3_sb = consts.tile([128, NM2, 3], f32r)
        ones31 = consts.tile([3, 1], f32r)
        ones13 = consts.tile([1, 3], f32r)
        ones1B = consts.tile([1, BS], f32r)
        idm = consts.tile([128, 128], f32)
        nln2 = consts.tile([128, 1], f32)
        half = consts.tile([128, 1], f32)
        nc.gpsimd.dma_start(idm[:], idm_d[:])
        nc.gpsimd.dma_start(b1_sb[:], b1_d[:])
        nc.gpsimd.dma_start(b2_sb[:], b2_d[:])
        nc.gpsimd.dma_start(b3_sb[:], b3_d[:])
        nc.gpsimd.dma_start(w3_sb[:], w3h_d[:])
        nc.any.memset(ones31[:].bitcast(f32), 1.0)
        nc.any.memset(ones13[:].bitcast(f32), 1.0)
        nc.any.memset(ones1B[:].bitcast(f32), 1.0)
        nc.any.memset(nln2[:], NEG_LN2)
        nc.any.memset(half[:], 0.5)

        # ---- z tiles: bf16 on arrival, packed fp8 (zl|zh per quarter) after
        # conversion. z8v(t) views tile t as [128, zn, NQ, 2, QC].
        zinfo = []
        k = 0
        for zn in ZGROUPS:
            zinfo.append((zn, k))
            k += zn
        zpools = {
            zn: ctx.enter_context(
                tc.tile_pool(name=f"z{zn}", bufs=sum(1 for g, _ in zinfo if g == zn))
            )
            for zn in sorted({g for g, _ in zinfo})
        }
        zidx_of = {}
        for gi, (zn, k0) in enumerate(zinfo):
            for i in range(zn):
                zidx_of[k0 + i] = gi
        z_tiles = []  # (tile, zn, base_k)
        z8vs = []

        def seam_split(k0, cnt):
            """Yield (tile_idx, local_k, n, off) segments covering chunks
            [k0, k0+cnt), where off is the offset within the caller range."""
            done = 0
            while done < cnt:
                gi = zidx_of[k0 + done]
                zt, zn, bk = z_tiles[gi]
                lk = k0 + done - bk
                take = min(cnt - done, zn - lk)
                yield gi, lk, take, done
                done += take

        yTs = ctx.enter_context(tc.tile_pool(name="yTs", bufs=2))
        ypl = ctx.enter_context(tc.tile_pool(name="ypl", bufs=2))
        lpp = ctx.enter_context(tc.tile_pool(name="lpp", bufs=2))
        z1p = ctx.enter_context(tc.tile_pool(name="z1p", bufs=4))
        w1p = ctx.enter_context(tc.tile_pool(name="w1p", bufs=4))
        z1st = ctx.enter_context(tc.tile_pool(name="z1st", bufs=3))
        w2p = ctx.enter_context(tc.tile_pool(name="w2p", bufs=3))
        cstack = ExitStack()
        wpp = cstack.enter_context(tc.tile_pool(name="wpp", bufs=NWPG))
        sselp = cstack.enter_context(tc.tile_pool(name="sselp", bufs=SELBUFS))
        psg = cstack.enter_context(tc.tile_pool(name="psg", bufs=1, space="PSUM"))
        gtp = cstack.enter_context(tc.tile_pool(name="gtp", bufs=GTBUFS))
        stp = cstack.enter_context(tc.tile_pool(name="stp", bufs=STBUFS))
        w1p0 = cstack.enter_context(tc.tile_pool(name="w1p0", bufs=NK // P0G + 1))
        psmI = ExitStack()
        psmIp = psmI.enter_context(tc.tile_pool(name="psmI", bufs=1, space="PSUM"))
        accs_p0 = (
            psmIp.tile([128, BS], f32, tag="p0a", name="acc0p0"),
            psmIp.tile([128, BS], f32, tag="p0b", name="acc1p0"),
        )
        accs_pA = (
            psmIp.tile([128, BS], f32, tag="pAa", name="acc0pA"),
            psmIp.tile([128, BS], f32, tag="pAb", name="acc1pA"),
        )
        bstack = ExitStack()
        pB = bstack.enter_context(tc.tile_pool(name="pB", bufs=1, space="PSUM"))

        rm_sb = consts.tile([50, 180], bf16)
        nc.gpsimd.dma_start(rm_sb[:], rmat_d[:])

        wpts = []
        yTc = [None] * NQ

        def emit_zgroup_dma(gi, h):
            zn, k0 = zinfo[gi]
            cols = slice(h * (BS // 2), (h + 1) * (BS // 2))
            if h == 0:
                zt = zpools[zn].tile([128, zn, BS], bf16, tag=f"z{zn}",
                                     name=f"zt{gi}")
                z_tiles.append((zt, zn, k0))
                z8vs.append(
                    zt[:].bitcast(fp8).rearrange(
                        "p k (q v c) -> p k q v c", q=NQ, v=2, c=QC
                    )
                )
            else:
                zt = z_tiles[gi][0]
            zq = nc.sync if (h == 0 and gi % 2 == 0) else nc.scalar
            zq.dma_start(zt[:, :, cols], xt_d[:, k0 : k0 + zn, cols])

        def emit_zgroup_pools(gi, h):
            zn, k0 = zinfo[gi]
            for jj in range(zn):
                kk = k0 + jj
                if h == 0 and kk % WPG == 0:
                    g = kk // WPG
                    cnt = min(WPG, NK - g * WPG)
                    wpt = wpp.tile([128, WPG, 50], bf16, tag="wp", name=f"wp{g}")
                    qeng = nc.sync if g == 0 else nc.gpsimd
                    qeng.dma_start(wpt[:, :cnt, :], wpool_d[g, :, :cnt, :])
                    wpts.append(wpt)
                for ci in range(2):
                    c = 2 * h + ci
                    if yTc[0] is None:
                        yTc[0] = pB.tile([128, 4, 64], f32, tag="yt",
                                         name="yTc")
                    nc.tensor.matmul(
                        yTc[0][:, c, 0:50],
                        z_tiles[gi][0][:, jj, c * QC : (c + 1) * QC],
                        wpts[kk // WPG][:, kk % WPG, :],
                        start=(kk == 0),
                        stop=(kk == NK - 1),
                        skip_group_check=True,
                    )

        def emit_B(q):
            """Pools -> T -> Lp' hi/lo fp8 stack for quarter q.

            Returns lpstk [128, 4, QC] fp8: k-tiles (Lph t0, Lph t1+pad,
            Lpl t0, Lpl t1+pad)."""
            t = yTs.tile([128, 50], f32, tag="yTs", name=f"yTsb{q}")
            nc.vector.tensor_copy(t[:], yTc[0][:, q, 0:50])
            tp = pB.tile([64, 128], f32, tag="tpTab", bufs=1, name=f"tp{q}")
            nc.tensor.transpose(tp[0:50, 0:128], t[:], idm[:])
            y_q = ypl.tile([50, QC], bf16, tag="yq", name=f"y_{q}")
            nc.scalar.activation(y_q[:], tp[0:50, 0:128], AF.Relu)
            Tab = pB.tile([128, 2, QC], f32, tag="tpTab", bufs=1, name=f"Tab{q}")
            Ta = Tab[:, 0, :]
            Tb = Tab[0:52, 1, :]
            nc.tensor.matmul(Ta, rm_sb[:, 0:128], y_q[:])
            nc.tensor.matmul(Tb, rm_sb[:, 128:180], y_q[:])
            lpstk = lpp.tile([128, 4, QC], fp8, tag="lps", name=f"lps{q}")
            lp16a = yTs.tile([128, QC], bf16, tag="lp16a", name=f"lp16a{q}")
            lp16b = yTs.tile([52, QC], bf16, tag="lp16b", name=f"lp16b{q}")
            ea = yTs.tile([128, QC], bf16, tag="ea", name=f"ea{q}")
            eb = yTs.tile([52, QC], bf16, tag="eb", name=f"eb{q}")
            # Lp' = softplus(-T) - ln2 = ln(0.5 + exp(-T - ln2))
            nc.scalar.activation(ea[:], Ta, AF.Exp, scale=-1.0,
                                 bias=nln2[:, 0:1])
            nc.scalar.activation(eb[:], Tb, AF.Exp, scale=-1.0,
                                 bias=nln2[0:52, 0:1])
            nc.scalar.activation(lp16a[:], ea[:], AF.Ln, bias=half[:, 0:1])
            nc.scalar.activation(lp16b[:], eb[:], AF.Ln, bias=half[0:52, 0:1])
            nc.vector.memset(lpstk[:, 1, :], 0.0)
            nc.vector.memset(lpstk[:, 3, :], 0.0)
            nc.scalar.activation(lpstk[:, 0, :], lp16a[:], AF.Copy)
            nc.scalar.activation(lpstk[0:52, 1, :], lp16b[:], AF.Copy)
            nc.vector.tensor_sub(lpstk[:, 2, :], lp16a[:], lpstk[:, 0, :])
            nc.vector.tensor_sub(lpstk[0:52, 3, :], lp16b[:], lpstk[0:52, 1, :])
            return lpstk

        # ---- pair-0 w1 prefetch (m-tiles 0/1), 8-chunk groups
        p0grp = [(P0G * j, min(P0G, NK - P0G * j)) for j in range((NK + P0G - 1) // P0G)]
        p0w1 = []

        def issue_p0(j):
            k0, cnt = p0grp[j]
            wtq = w1p0.tile([128, P0G, 2, 256], fp8, tag="w1q0", name=f"w1q0_{j}")
            nc.sync.dma_start(wtq[:, :cnt, :, :], w1q_d[0, :, k0 : k0 + cnt, :, :])
            p0w1.append(wtq)

        sls = (slice(0, 128), slice(128, 256))

        def consume_p0_group(k0, cnt, h, accs, started):
            """Pair-0 matmuls over chunks [k0,k0+cnt) for column half h."""
            hcols = slice(h * 2 * QC, (h + 1) * 2 * QC)
            qr = slice(2 * h, 2 * h + 2)
            for gi, lk, n, off in seam_split(k0, cnt):
                z8v = z8vs[gi]
                for jj in range(0, n, 2):
                    kk = k0 + off + jj
                    wt = p0w1[kk // P0G]
                    lw = kk - (kk // P0G) * P0G
                    rhs1 = z8v[:, lk + jj : lk + jj + 2, qr, 1, :]
                    for mi in range(2):
                        nc.tensor.matmul(
                            accs[mi][:, hcols],
                            wt[:, lw : lw + 2, 0, sls[mi]],
                            rhs1,
                            start=not started[mi],
                            stop=False,
                            perf_mode=DR,
                            skip_group_check=True,
                        )
                        started[mi] = True
                for jj in range(n):
                    kk = k0 + off + jj
                    wt = p0w1[kk // P0G]
                    lw = kk - (kk // P0G) * P0G
                    rhs2 = z8v[:, lk + jj, qr, :, :].transpose([0, 2, 1, 3])
                    last = kk == NK - 1
                    for mi in range(2):
                        nc.tensor.matmul(
                            accs[mi][:, hcols],
                            wt[:, lw, :, sls[mi]],
                            rhs2,
                            start=False,
                            stop=last,
                            perf_mode=DR,
                            skip_group_check=True,
                        )

        # ---- phase A: stream x (bf16) by halves, pools per quarter.
        # h0 DMA + pools now; p0 + h1 DMAs issued up front (h1 pools are
        # woven into C-h0 via hooks once the data has landed).
        for gi in range(len(zinfo)):
            emit_zgroup_dma(gi, 0)
            emit_zgroup_pools(gi, 0)
        for j in range(len(p0grp)):
            issue_p0(j)
        for gi in range(len(zinfo)):
            emit_zgroup_dma(gi, 1)

        selgrp = [(SELG * g, min(SELG, NK - SELG * g)) for g in range(NSG)]
        sel_tiles = {}

        def convert_group(q, sg, k0, cnt, lpstk, st):
            """Sel matmuls + gate exp + in-place fp8 packing for one
            chunk-group of one column quarter."""
            qcols = slice(q * QC, (q + 1) * QC)
            gq = psg.tile([128, SELG, QC], f32, tag="gq", bufs=2,
                          name=f"gq{q}_{sg}")
            for j in range(cnt):
                nc.tensor.matmul(
                    gq[:, j, :], st[:, j, :, :], lpstk[:, 0:2, :],
                    start=True, stop=False, perf_mode=DR,
                    skip_group_check=True,
                )
                nc.tensor.matmul(
                    gq[:, j, :], st[:, j, :, :], lpstk[:, 2:4, :],
                    start=False, stop=True, perf_mode=DR,
                    skip_group_check=True,
                )
            # gt4 = 4*G = exp(-R - ln2)
            gt4 = gtp.tile([128, SELG, QC], bf16, tag="gt", name=f"gt{q}_{sg}")
            nc.scalar.activation(gt4[:, :cnt, :], gq[:, :cnt, :], AF.Exp,
                                 scale=-1.0, bias=nln2[:, 0:1])
            stage = stp.tile([128, SELG, QC], bf16, tag="st",
                             name=f"st{q}_{sg}")
            for gi, lk, n, off in seam_split(k0, cnt):
                zt = z_tiles[gi][0]
                z8v = z8vs[gi]
                sseg = stage[:, off : off + n, :]
                nc.vector.tensor_mul(sseg, zt[:, lk : lk + n, qcols],
                                     gt4[:, off : off + n, :])
                zh = z8v[:, lk : lk + n, q, 1, :]
                zl = z8v[:, lk : lk + n, q, 0, :]
                if (2 * sg + q) % 5 < 3:
                    nc.vector.tensor_copy(zh, sseg)
                else:
                    nc.scalar.activation(zh, sseg, AF.Copy)
                sub_eng = nc.gpsimd if (2 * sg + q) % 2 == 0 else nc.vector
                sub_eng.tensor_sub(zl, sseg, zh)

        def issue_sel(sg):
            k0, cnt = selgrp[sg]
            st = sselp.tile([128, SELG, 2, 128], fp8, tag="sq",
                            name=f"sq{sg}")
            nc.sync.dma_start(st[:, :cnt, :, :], ssel_d[sg, :, :cnt, :, :])
            return st

        def emit_C(h, lpstks, hook, p0_started):
            """Gates + packing for both quarters of half h, chasing pair-0."""
            p0_backlog = []
            sts = {sg: issue_sel(sg) for sg in range(2)}
            for sg, (k0, cnt) in enumerate(selgrp):
                if sg + 2 < NSG:
                    sts[sg + 2] = issue_sel(sg + 2)
                st = sts.pop(sg)
                for q in (2 * h, 2 * h + 1):
                    convert_group(q, sg, k0, cnt, lpstks[q - 2 * h], st)
                p0_backlog.append((k0, cnt))
                if len(p0_backlog) > 2:
                    bk0, bcnt = p0_backlog.pop(0)
                    consume_p0_group(bk0, bcnt, h, accs_p0, p0_started)
                hook(h, sg)
            for bk0, bcnt in p0_backlog:
                consume_p0_group(bk0, bcnt, h, accs_p0, p0_started)

        mm2_pending = []

        w2b_sb = [None]

        def prepare_w2b(mt):
            # bf16 copy (hi+lo recombined) of the final pair's w2 slice, at
            # the fp8 scale so it accumulates into the same PSUM group
            w2t = w2p.tile([128, 2, 2, NM2, 128], fp8, tag="w2", name="w2l")
            nc.scalar.dma_start(w2t[:], w2k_d[mt // 2])
            w2b = w2bp.tile([128, 2, NM2, 128], bf16, tag="w2b", name="w2b")
            nc.vector.tensor_add(w2b[:], w2t[:, :, 0, :, :], w2t[:, :, 1, :, :])
            w2b_sb[0] = w2b

        def pack_z1(mt, accs, last=False):
            """z1 pair (m-tiles mt, mt+1) -> fp8 (zl|zh) planes. The final
            pair stays bf16 (its mm2 runs immediately; skipping the packing
            chain shortens the tail's critical path)."""
            if last:
                z1t = z1p.tile([128, 2, BS], bf16, tag="z1l", name=f"z1_{mt}")
                for mi in range(2):
                    k2 = mt + mi
                    nc.scalar.activation(z1t[:, mi, :], accs[mi][:], AF.Relu,
                                         scale=S_Z1 / S1,
                                         bias=b1_sb[:, k2 : k2 + 1])
            else:
                z1t = z1p.tile([128, 2, 2, BS], fp8, tag="z1", name=f"z1_{mt}")
                for mi in range(2):
                    k2 = mt + mi
                    stg = z1st.tile([128, BS], bf16, tag="z1s", name=f"z1s{k2}")
                    nc.scalar.activation(stg[:], accs[mi][:], AF.Relu,
                                         scale=S_Z1 / S1,
                                         bias=b1_sb[:, k2 : k2 + 1])
                    if mi == 0:
                        nc.vector.tensor_copy(z1t[:, mi, 1, :], stg[:])
                    else:
                        nc.scalar.activation(z1t[:, mi, 1, :], stg[:], AF.Copy)
                    nc.vector.tensor_sub(z1t[:, mi, 0, :], stg[:],
                                         z1t[:, mi, 1, :])
            if last:
                mm2_pending.append((mt // 2, z1t, None, last))
            else:
                w2t = w2p.tile([128, 2, 2, NM2, 128], fp8, tag="w2",
                               name=f"w2_{mt}")
                nc.scalar.dma_start(w2t[:], w2k_d[mt // 2])
                mm2_pending.append((mt // 2, z1t, w2t, last))

        def emit_mm2(m2_major=False):
            order = (
                [(e, m2) for m2 in range(NM2) for e in mm2_pending]
                if m2_major else
                [(e, m2) for e in mm2_pending for m2 in range(NM2)]
            )
            for (g2, z1t, w2t, last), m2 in order:
                if last:
                    for i in range(2):
                        nc.tensor.matmul(
                            acc2s[m2][:],
                            w2b_sb[0][:, i, m2, :],
                            z1t[:, i, :],
                            start=False,
                            stop=(g2 == NK2 // 2 - 1 and i == 1),
                            skip_group_check=True,
                        )
                    continue
                nc.tensor.matmul(
                    acc2s[m2][:],
                    w2t[:, :, 0, m2, :],
                    z1t[:, :, 1, :],
                    start=(g2 == 0),
                    stop=False,
                    perf_mode=DR,
                    skip_group_check=True,
                )
                for i in range(2):
                    nc.tensor.matmul(
                        acc2s[m2][:],
                        w2t[:, i, :, m2, :],
                        z1t[:, i, :, :],
                        start=False,
                        stop=(g2 == NK2 // 2 - 1 and i == 1),
                        perf_mode=DR,
                        skip_group_check=True,
                    )
            mm2_pending.clear()

        # ---- w1 streaming machinery (shared by C-h1 pair-1 and phase D)
        nkg = (NK + KG - 1) // KG
        kgrp = [(g * KG, min(KG, NK - g * KG)) for g in range(nkg)]
        w1_fifo = {}

        def issue_w1(mt, k0, cnt):
            wtq = w1p.tile([128, KG, 2, 256], fp8, tag="w1q")
            nc.sync.dma_start(wtq[:, :cnt, :, :],
                              w1q_d[mt // 2, :, k0 : k0 + cnt, :, :])
            w1_fifo.setdefault(mt, []).append((wtq, k0, cnt))

        def consume_w1(mt, accs):
            wtq, k0, cnt = w1_fifo[mt].pop(0)
            for gi, lk, n, off in seam_split(k0, cnt):
                z8v = z8vs[gi]
                for jj in range(0, n, 2):
                    kk = k0 + off + jj
                    rhs1 = z8v[:, lk + jj : lk + jj + 2, :, 1, :]
                    for mi in range(2):
                        nc.tensor.matmul(
                            accs[mi][:],
                            wtq[:, off + jj : off + jj + 2, 0, sls[mi]],
                            rhs1,
                            start=(kk == 0),
                            stop=False,
                            perf_mode=DR,
                            skip_group_check=True,
                        )
                for jj in range(n):
                    kk = k0 + off + jj
                    rhs2 = z8v[:, lk + jj, :, :, :].transpose([0, 2, 1, 3])
                    for mi in range(2):
                        nc.tensor.matmul(
                            accs[mi][:],
                            wtq[:, off + jj, :, sls[mi]],
                            rhs2,
                            start=False,
                            stop=(kk == NK - 1),
                            perf_mode=DR,
                            skip_group_check=True,
                        )

        def consume_w1_half(mt, accs, h, started):
            """Consume one streamed w1 group for column half h only."""
            wtq, k0, cnt = w1_fifo[mt].pop(0)
            hcols = slice(h * 2 * QC, (h + 1) * 2 * QC)
            qr = slice(2 * h, 2 * h + 2)
            for gi, lk, n, off in seam_split(k0, cnt):
                z8v = z8vs[gi]
                for jj in range(0, n, 2):
                    rhs1 = z8v[:, lk + jj : lk + jj + 2, qr, 1, :]
                    for mi in range(2):
                        nc.tensor.matmul(
                            accs[mi][:, hcols],
                            wtq[:, off + jj : off + jj + 2, 0, sls[mi]],
                            rhs1,
                            start=not started[mi],
                            stop=False,
                            perf_mode=DR,
                            skip_group_check=True,
                        )
                        started[mi] = True
                for jj in range(n):
                    kk = k0 + off + jj
                    rhs2 = z8v[:, lk + jj, qr, :, :].transpose([0, 2, 1, 3])
                    for mi in range(2):
                        nc.tensor.matmul(
                            accs[mi][:, hcols],
                            wtq[:, off + jj, :, sls[mi]],
                            rhs2,
                            start=False,
                            stop=(kk == NK - 1),
                            perf_mode=DR,
                            skip_group_check=True,
                        )

        # ---- phases B/C. C-h0 carries: pools-h1, pair-A's h0 columns.
        # C-h1 carries: pair-A's h1 columns (w1 re-streamed) + pair-B full.
        h1_state = [0]
        pA_state = [0]
        pA_started = [False, False]

        def advance_pA(h):
            g = pA_state[0]
            if g + 1 < nkg:
                issue_w1(2, *kgrp[g + 1])
            consume_w1_half(2, accs_pA, h, pA_started)
            pA_state[0] += 1

        def hook_h0(h, sg):
            if sg >= 2:
                frontier = SELG * (sg - 2) + selgrp[sg - 2][1]
                while pA_state[0] < nkg:
                    g = pA_state[0]
                    # pace to the w1 DMA stream (it contends with xt-h1 and
                    # the pair-0 prefetch): don't enqueue PE work whose
                    # weights can't have landed yet
                    if sg < 8 + (g * 3) // 2:
                        break
                    if kgrp[g][0] + kgrp[g][1] > frontier:
                        break
                    advance_pA(0)
            if sg >= 14 and h1_state[0] < len(zinfo):
                for _ in range(3):
                    if h1_state[0] < len(zinfo):
                        emit_zgroup_pools(h1_state[0], 1)
                        h1_state[0] += 1

        lps0 = emit_B(0)
        lps1 = emit_B(1)
        issue_w1(2, *kgrp[0])
        emit_C(0, (lps0, lps1), hook_h0, [False, False])
        pA_h0_left = list(range(pA_state[0], nkg))
        while h1_state[0] < len(zinfo):
            emit_zgroup_pools(h1_state[0], 1)
            h1_state[0] += 1
        lps2 = emit_B(2)
        lps3 = emit_B(3)

        bstack.close()
        astack = ExitStack()
        psmD = astack.enter_context(tc.tile_pool(name="psmD", bufs=1, space="PSUM"))
        accs_pB = (
            psmD.tile([128, BS], f32, tag="pBa", name="acc0pB"),
            psmD.tile([128, BS], f32, tag="pBb", name="acc1pB"),
        )
        pA_h0_started = pA_started

        def advance_pA_h0flush():
            g = pA_state[0]
            if g + 1 < nkg:
                issue_w1(2, *kgrp[g + 1])
            consume_w1_half(2, accs_pA, 0, pA_h0_started)
            pA_state[0] += 1

        pA1_state = [0]
        pA_started = [False, False]
        pB_state = [0]
        pB_started = [False, False]
        issue_w1(4, *kgrp[0])

        pA1_fifo_seeded = [False]

        def advance_pA1():
            if not pA1_fifo_seeded[0]:
                issue_w1(2, *kgrp[0])
                pA1_fifo_seeded[0] = True
            g = pA1_state[0]
            if g + 1 < nkg:
                issue_w1(2, *kgrp[g + 1])
            consume_w1_half(2, accs_pA, 1, pA_started)
            pA1_state[0] += 1

        def advance_pB():
            g = pB_state[0]
            if g + 1 < nkg:
                issue_w1(4, *kgrp[g + 1])
            consume_w1(4, accs_pB)
            pB_state[0] += 1

        def hook_h1(h, sg):
            if sg < 2:
                return
            frontier = SELG * (sg - 2) + selgrp[sg - 2][1]
            while True:
                if pA_h0_left:
                    pA_h0_left.pop(0)
                    advance_pA_h0flush()
                    continue
                ja = pA1_state[0]
                jb = pB_state[0]
                if ja < nkg and (jb >= nkg or ja <= jb):
                    if kgrp[ja][0] + kgrp[ja][1] > frontier:
                        break
                    advance_pA1()
                elif jb < nkg:
                    if kgrp[jb][0] + kgrp[jb][1] > frontier:
                        break
                    advance_pB()
                else:
                    break

        emit_C(1, (lps2, lps3), hook_h1, [False, False])
        while pA_h0_left:
            pA_h0_left.pop(0)
            advance_pA_h0flush()
        while pA1_state[0] < nkg:
            advance_pA1()
        while pB_state[0] < nkg:
            advance_pB()

        # ---- evict pairs 0, A, B; start mm2 accumulators
        pack_z1(0, accs_p0)
        pack_z1(2, accs_pA)
        pack_z1(4, accs_pB)
        astack.close()
        psmI.close()
        cstack.close()

        dstack = ExitStack()
        psm = dstack.enter_context(tc.tile_pool(name="psm", bufs=1, space="PSUM"))
        w2bp = dstack.enter_context(tc.tile_pool(name="w2bp", bufs=1))
        e2stack = ExitStack()
        psm2 = e2stack.enter_context(tc.tile_pool(name="psm2", bufs=1, space="PSUM"))
        acc2s = [
            psm2.tile([128, BS], f32, tag=f"mm2_{m2}", name=f"acc2_{m2}")
            for m2 in range(NM2)
        ]

        # ---- phase D: pairs 3..15 full-width, mm2 consumed mid-pair
        issue_w1(6, *kgrp[0])
        for mt in range(6, NM1, 2):
            if mt == NM1 - 2:
                prepare_w2b(mt)
            acc0 = psm.tile([128, BS], f32, tag="mm1a", bufs=2, name=f"acc0_{mt}")
            acc1 = psm.tile([128, BS], f32, tag="mm1b", bufs=1, name=f"acc1_{mt}")
            for g in range(nkg):
                if g + 1 < nkg:
                    issue_w1(mt, *kgrp[g + 1])
                elif mt + 2 < NM1:
                    issue_w1(mt + 2, *kgrp[0])
                consume_w1(mt, (acc0, acc1))
                if g == nkg // 2:
                    emit_mm2()
            pack_z1(mt, (acc0, acc1), last=(mt == NM1 - 2))
        emit_mm2(m2_major=True)

        # ---- phase E: z2 = relu(acc2/S2 + b2) feeding fused mm3
        with (
            tc.tile_pool(name="z2p", bufs=2) as z2p,
            tc.tile_pool(name="tailp", bufs=1) as tailp,
        ):
            acc3 = psm2.tile([3, BS], f32, tag="f")
            for m2 in range(NM2):
                z2t = z2p.tile([128, BS], f32r, tag="z2")
                nc.scalar.activation(
                    z2t[:], acc2s[m2][:], AF.Relu, scale=1.0 / S2,
                    bias=b2_sb[:, m2 : m2 + 1]
                )
                nc.tensor.matmul(
                    acc3[:], w3_sb[:, m2, :], z2t[:],
                    start=(m2 == 0), stop=False,
                    skip_group_check=True,
                )
            nc.tensor.matmul(
                acc3[:], b3_sb[:], ones1B[:],
                start=False, stop=True,
                skip_group_check=True,
            )
            # ---- phase F: out = z3 / ||z3||, with 1/sqrt(s) computed as
            # exp(-0.5 ln s) + one Newton step (avoids the Sqrt act-table
            # load; exp/ln/square live in the already-loaded table)
            sq = tailp.tile([3, BS], f32r)
            nc.scalar.activation(sq[:], acc3[:], AF.Square)
            z3sb = tailp.tile([3, BS], f32)
            nc.scalar.copy(z3sb[:], acc3[:])
            sps = psm2.tile([128, BS], f32, tag="mm2_0", name="sps")
            nc.tensor.matmul(sps[0:1, :], ones31[:], sq[:])
            rts = tailp.tile([1, BS], f32)
            nc.scalar.activation(rts[:], sps[0:1, :], AF.Sqrt)
            inv = tailp.tile([1, BS], f32r)
            with nc.allow_low_precision(reason="f32r is fp32-width storage"):
                nc.vector.reciprocal(inv[:], rts[:])
            inv3 = psm2.tile([128, BS], f32, tag="mm2_1", name="inv3")
            nc.tensor.matmul(inv3[0:3, :], ones13[:], inv[:])
            outt = tailp.tile([3, BS], f32)
            nc.vector.tensor_mul(outt[:], z3sb[:], inv3[0:3, :])
            nc.sync.dma_start(out_d[:], outt[:])
        e2stack.close()
        dstack.close()

    nc.compile()
    return nc


def _prep_shared(conv_w, F_w, w1, b1, w2, b2, w3, b3):
    """Host-side weight layouts shared by all cores."""
    fa = np.arange(F)
    c_idx = fa // 4000
    d_idx = (fa // 400) % 10
    h_idx = (fa // 40) % 10
    w_idx = fa % 40

    # pooled conv: y_pre[j] = sum_f wp[f, j] * x^T[f, b]
    wp = np.zeros((NWPG * WPG * 128, 50), np.float32)
    wp[fa, h_idx] = conv_w[c_idx] / 400.0
    wp[fa, 10 + w_idx] = conv_w[c_idx] / 100.0
    wpool = np.ascontiguousarray(
        wp.reshape(NWPG, WPG, 128, 50).transpose(0, 2, 1, 3)
    ).astype(np_bf16)

    # rmat: T[r, b] = sum_j rmat[j, r] * y[j, b]
    rm = np.zeros((50, 180), np.float32)
    cc10 = np.repeat(np.arange(3), 10)
    rm[np.tile(np.arange(10), 3), np.arange(30)] = F_w[cc10]
    rm[np.tile(np.arange(10), 3), 30 + np.arange(30)] = F_w[cc10]
    cc40 = np.repeat(np.arange(3), 40)
    rm[10 + np.tile(np.arange(40), 3), 60 + np.arange(120)] = F_w[cc40]

    # selection: logG[f] = sel[f, :] @ Lp'  (exact 0/1 weights, fp8)
    sel = np.zeros((FP, 180), np.float32)
    sel[fa, c_idx * 10 + d_idx] = 1.0
    sel[fa, 30 + c_idx * 10 + h_idx] = 1.0
    sel[fa, 60 + c_idx * 40 + w_idx] = 1.0
    sel_full = np.zeros((NSG * SELG * 128, 256), np.float32)
    sel_full[:FP, :180] = sel
    ssel = np.ascontiguousarray(
        sel_full.reshape(NSG, SELG, 128, 2, 128).transpose(0, 4, 1, 3, 2)
    ).astype(np_fp8)

    # w1: fp8 hi/lo pair, scaled by S_W1
    w1p_ = np.zeros((H1, FP), np.float32)
    w1p_[:, :F] = w1
    wt = w1p_.T * S_W1  # [FP, H1]
    wh8 = wt.astype(np_fp8)
    wl8 = (wt - wh8.astype(np.float32)).astype(np_fp8)
    w1q = np.zeros((NM1 // 2, 128, NK, 2, 256), np_fp8)
    w1q[:, :, :, 0, :] = wh8.reshape(NK, 128, NM1 // 2, 256).transpose(2, 1, 0, 3)
    w1q[:, :, :, 1, :] = wl8.reshape(NK, 128, NM1 // 2, 256).transpose(2, 1, 0, 3)

    # w2: fp8 hi/lo pair, scaled by S_W2
    w2t_ = w2.T * S_W2  # [H1, H2]
    wh2 = w2t_.astype(np_fp8)
    wl2 = (w2t_ - wh2.astype(np.float32)).astype(np_fp8)
    w2k = np.zeros((NK2 // 2, 128, 2, 2, NM2, 128), np_fp8)
    w2k[:, :, :, 0, :, :] = wh2.reshape(NK2 // 2, 2, 128, NM2, 128).transpose(
        0, 2, 1, 3, 4
    )
    w2k[:, :, :, 1, :, :] = wl2.reshape(NK2 // 2, 2, 128, NM2, 128).transpose(
        0, 2, 1, 3, 4
    )

    w3h = np.ascontiguousarray(w3.reshape(3, NM2, 128).transpose(2, 1, 0))

    return {
        "wpool": wpool,
        "rmat": rm.astype(np_bf16),
        "ssel": ssel,
        "w1q": w1q,
        "w2k": w2k,
        "w3h": w3h,
        "b1g4": np.ascontiguousarray((S_Z1 * b1).reshape(NM1, 128).T),
        "b2g": np.ascontiguousarray(b2.reshape(NM2, 128).T),
        "b3r": np.ascontiguousarray(b3.reshape(1, 3)),
        "idm": np.eye(128, dtype=np.float32),
    }


def make_in_maps(x, conv_w, F_w, w1, b1, w2, b2, w3, b3):
    x = np.asarray(x, np.float32).reshape(B_TOT, F)
    shared = _prep_shared(
        np.asarray(conv_w, np.float32).reshape(3),
        np.asarray(F_w, np.float32).reshape(3),
        np.asarray(w1, np.float32),
        np.asarray(b1, np.float32),
        np.asarray(w2, np.float32),
        np.asarray(b2, np.float32),
        np.asarray(w3, np.float32),
        np.asarray(b3, np.float32),
    )
    in_maps = []
    for c in range(N_CORES):
        xs = x[c * BS : (c + 1) * BS]
        xt = np.zeros((FP, BS), np_bf16)
        xt[:F] = xs.T.astype(np_bf16)
        m = dict(shared)
        m["xt"] = np.ascontiguousarray(
            xt.reshape(NK, 128, BS).transpose(1, 0, 2)
        )
        in_maps.append(m)
    return in_maps


def get_nc():
    if "nc" not in _NC_CACHE:
        _NC_CACHE["nc"] = build_nc()
    return _NC_CACHE["nc"]


def kernel(**inputs) -> np.ndarray:
    nc = get_nc()
    in_maps = make_in_maps(**inputs)
    res = run_bass_kernel_spmd(nc, in_maps, core_ids=list(range(N_CORES)))
    out = np.concatenate([r["out"] for r in res.results], axis=1)  # [3, 4096]
    return np.ascontiguousarray(out.T, dtype=np.float32)


# revision 43
# speedup vs baseline: 1.0014x; 1.0014x over previous
"""Trainium2 Bass kernel for nn_CABlock_26912265077025.

Architecture: CA-gating block (pools -> conv -> sigmoid gates -> x*gd*gh*gw)
followed by a 12000->4096->512->3 MLP and row L2-normalization.

Strategy: pure data parallelism over the batch across 8 NeuronCores
(512 rows each). The dominant 12032x4096 GEMM (and the 4096x512 GEMM) run
as fp8e4 DoubleRow matmuls (2 k-tiles / 0.5 cycles per row on the PE) using
a 3-term residual decomposition that keeps bf16-level accuracy:
  z = zh + zl, w = wh + wl (all fp8, residuals at the same scale)
  y ~= zh@wh + (zl@wh + zh@wl)   [the zl@wl term is ~0.1% and dropped]
The two cross terms are computed in a single DoubleRow pass that pairs the
(zl, zh) planes with (wh, wl) planes along the contraction dim, so mm1
costs 0.75x its bf16 PE time at the 4x fp8 rate.

The gated activations are quantized on the fly, packed IN PLACE over the
dead bf16 x bytes per column-quarter (zl/zh planes of 128 bytes each), via
a 4-pass chain: exp (ACT) -> gate-mul (DVE) -> fp8 copy (DVE/ACT alternate)
-> subtract with direct fp8 output (DVE). Gate selection matmuls also run
as fp8 DoubleRow using exact 0/1 selection weights and a mean-shifted
softplus pair (Lp' = softplus(-T) - ln2, hi+lo fp8), with the e^{-3ln2}
constant folded into the gate exp bias.
"""

from contextlib import ExitStack

import ml_dtypes
import numpy as np

import concourse.bass as bass
import concourse.mybir as mybir
import concourse.tile as tile
from concourse import bacc
from concourse.bass_utils import run_bass_kernel_spmd

N_CORES = 8
B_TOT = 4096
BS = B_TOT // N_CORES            # 512 batch rows per core
F = 12000                        # 3*10*10*40 flattened features
NK = 94                          # ceil(F/128)
FP = NK * 128                    # 12032 (rows F..FP-1 zero-padded)
H1, H2 = 4096, 512
NM1 = H1 // 128                  # 32 mm1 output tiles
NK2, NM2 = H1 // 128, H2 // 128  # 32, 4
KG = 6                           # w1 k-chunks per DMA group in phase D
SELG = 4                         # sel/conversion chunk-group size
NSG = (NK + SELG - 1) // SELG    # 24 groups (last has 2)
WPG = 8                          # wpool chunks per group
NWPG = (NK + WPG - 1) // WPG
P0G = 8                          # pair-0 w1 prefetch chunk-group size
NQ = 4                           # column quarters
QC = BS // NQ                    # 128 cols per quarter

S_Z = 4.0                        # z scale (z4 = 4*x*G)
S_W1 = 1024.0
S1 = S_Z * S_W1                  # 4096
S_Z1 = 4.0
S_W2 = 512.0
S2 = S_Z1 * S_W2                 # 2048
NEG_LN2 = -0.6931471805599453

f32 = mybir.dt.float32
f32r = mybir.dt.float32r
bf16 = mybir.dt.bfloat16
fp8 = mybir.dt.float8e4
np_bf16 = ml_dtypes.bfloat16
np_fp8 = ml_dtypes.float8_e4m3
AF = mybir.ActivationFunctionType
DR = mybir.MatmulPerfMode.DoubleRow

_NC_CACHE = {}

# z tile chunk grouping (all even so chunk-pairs never straddle tiles)
ZGROUPS = [2, 2] + [4] * 22 + [2]
assert sum(ZGROUPS) == NK


def build_nc(W1BUFS=4, SELBUFS=3, GTBUFS=3, STBUFS=3):
    nc = bacc.Bacc(None, target_bir_lowering=False)

    xt_d = nc.dram_tensor("xt", [128, NK, BS], bf16, kind="ExternalInput")
    w1q_d = nc.dram_tensor("w1q", [NM1 // 2, 128, NK, 2, 256], fp8,
                           kind="ExternalInput")
    wpool_d = nc.dram_tensor("wpool", [NWPG, 128, WPG, 50], bf16,
                             kind="ExternalInput")
    rmat_d = nc.dram_tensor("rmat", [50, 180], bf16, kind="ExternalInput")
    ssel_d = nc.dram_tensor("ssel", [NSG, 128, SELG, 2, 128], fp8,
                            kind="ExternalInput")
    w2k_d = nc.dram_tensor("w2k", [NK2 // 2, 128, 2, 2, NM2, 128], fp8,
                           kind="ExternalInput")
    w3h_d = nc.dram_tensor("w3h", [128, NM2, 3], f32r, kind="ExternalInput")
    b1_d = nc.dram_tensor("b1g4", [128, NM1], f32, kind="ExternalInput")
    b2_d = nc.dram_tensor("b2g", [128, NM2], f32, kind="ExternalInput")
    b3_d = nc.dram_tensor("b3r", [1, 3], f32r, kind="ExternalInput")
    idm_d = nc.dram_tensor("idm", [128, 128], f32, kind="ExternalInput")
    out_d = nc.dram_tensor("out", [3, BS], f32, kind="ExternalOutput")

    with tile.TileContext(nc) as tc, ExitStack() as ctx:
        consts = ctx.enter_context(tc.tile_pool(name="consts", bufs=1))

        b1_sb = consts.tile([128, NM1], f32)
        b2_sb = consts.tile([128, NM2], f32)
        b3_sb = consts.tile([1, 3], f32r)
        w3_sb = consts.tile([128, NM2, 3], f32r)
        ones31 = consts.tile([3, 1], f32r)
        ones13 = consts.tile([1, 3], f32r)
        ones1B = consts.tile([1, BS], f32r)
        idm = consts.tile([128, 128], f32)
        nln2 = consts.tile([128, 1], f32)
        half = consts.tile([128, 1], f32)
        nc.gpsimd.dma_start(idm[:], idm_d[:])
        nc.gpsimd.dma_start(b1_sb[:], b1_d[:])
        nc.gpsimd.dma_start(b2_sb[:], b2_d[:])
        nc.gpsimd.dma_start(b3_sb[:], b3_d[:])
        nc.gpsimd.dma_start(w3_sb[:], w3h_d[:])
        nc.any.memset(ones31[:].bitcast(f32), 1.0)
        nc.any.memset(ones13[:].bitcast(f32), 1.0)
        nc.any.memset(ones1B[:].bitcast(f32), 1.0)
        nc.any.memset(nln2[:], NEG_LN2)
        nc.any.memset(half[:], 0.5)

        # ---- z tiles: bf16 on arrival, packed fp8 (zl|zh per quarter) after
        # conversion. z8v(t) views tile t as [128, zn, NQ, 2, QC].
        zinfo = []
        k = 0
        for zn in ZGROUPS:
            zinfo.append((zn, k))
            k += zn
        zpools = {
            zn: ctx.enter_context(
                tc.tile_pool(name=f"z{zn}", bufs=sum(1 for g, _ in zinfo if g == zn))
            )
            for zn in sorted({g for g, _ in zinfo})
        }
        zidx_of = {}
        for gi, (zn, k0) in enumerate(zinfo):
            for i in range(zn):
                zidx_of[k0 + i] = gi
        z_tiles = []  # (tile, zn, base_k)
        z8vs = []

        def seam_split(k0, cnt):
            """Yield (tile_idx, local_k, n, off) segments covering chunks
            [k0, k0+cnt), where off is the offset within the caller range."""
            done = 0
            while done < cnt:
                gi = zidx_of[k0 + done]
                zt, zn, bk = z_tiles[gi]
                lk = k0 + done - bk
                take = min(cnt - done, zn - lk)
                yield gi, lk, take, done
                done += take

        yTs = ctx.enter_context(tc.tile_pool(name="yTs", bufs=2))
        ypl = ctx.enter_context(tc.tile_pool(name="ypl", bufs=2))
        lpp = ctx.enter_context(tc.tile_pool(name="lpp", bufs=2))
        z1p = ctx.enter_context(tc.tile_pool(name="z1p", bufs=4))
        w1p = ctx.enter_context(tc.tile_pool(name="w1p", bufs=4))
        z1st = ctx.enter_context(tc.tile_pool(name="z1st", bufs=3))
        w2p = ctx.enter_context(tc.tile_pool(name="w2p", bufs=3))
        cstack = ExitStack()
        wpp = cstack.enter_context(tc.tile_pool(name="wpp", bufs=NWPG))
        sselp = cstack.enter_context(tc.tile_pool(name="sselp", bufs=SELBUFS))
        psg = cstack.enter_context(tc.tile_pool(name="psg", bufs=1, space="PSUM"))
        gtp = cstack.enter_context(tc.tile_pool(name="gtp", bufs=GTBUFS))
        stp = cstack.enter_context(tc.tile_pool(name="stp", bufs=STBUFS))
        w1p0 = cstack.enter_context(tc.tile_pool(name="w1p0", bufs=NK // P0G + 1))
        psmI = ExitStack()
        psmIp = psmI.enter_context(tc.tile_pool(name="psmI", bufs=1, space="PSUM"))
        accs_p0 = (
            psmIp.tile([128, BS], f32, tag="p0a", name="acc0p0"),
            psmIp.tile([128, BS], f32, tag="p0b", name="acc1p0"),
        )
        accs_pA = (
            psmIp.tile([128, BS], f32, tag="pAa", name="acc0pA"),
            psmIp.tile([128, BS], f32, tag="pAb", name="acc1pA"),
        )
        bstack = ExitStack()
        pB = bstack.enter_context(tc.tile_pool(name="pB", bufs=1, space="PSUM"))

        rm_sb = consts.tile([50, 180], bf16)
        nc.gpsimd.dma_start(rm_sb[:], rmat_d[:])

        wpts = []
        yTc = [None] * NQ

        def emit_zgroup_dma(gi, h):
            zn, k0 = zinfo[gi]
            cols = slice(h * (BS // 2), (h + 1) * (BS // 2))
            if h == 0:
                zt = zpools[zn].tile([128, zn, BS], bf16, tag=f"z{zn}",
                                     name=f"zt{gi}")
                z_tiles.append((zt, zn, k0))
                z8vs.append(
                    zt[:].bitcast(fp8).rearrange(
                        "p k (q v c) -> p k q v c", q=NQ, v=2, c=QC
                    )
                )
            else:
                zt = z_tiles[gi][0]
            zq = nc.sync if (h == 0 and gi % 2 == 0) else nc.scalar
            zq.dma_start(zt[:, :, cols], xt_d[:, k0 : k0 + zn, cols])

        def emit_zgroup_pools(gi, h):
            zn, k0 = zinfo[gi]
            for jj in range(zn):
                kk = k0 + jj
                if h == 0 and kk % WPG == 0:
                    g = kk // WPG
                    cnt = min(WPG, NK - g * WPG)
                    wpt = wpp.tile([128, WPG, 50], bf16, tag="wp", name=f"wp{g}")
                    qeng = nc.sync if g == 0 else nc.gpsimd
                    qeng.dma_start(wpt[:, :cnt, :], wpool_d[g, :, :cnt, :])
                    wpts.append(wpt)
                for ci in range(2):
                    c = 2 * h + ci
                    if yTc[0] is None:
                        yTc[0] = pB.tile([128, 4, 64], f32, tag="yt",
                                         name="yTc")
                    nc.tensor.matmul(
                        yTc[0][:, c, 0:50],
                        z_tiles[gi][0][:, jj, c * QC : (c + 1) * QC],
                        wpts[kk // WPG][:, kk % WPG, :],
                        start=(kk == 0),
                        stop=(kk == NK - 1),
                        skip_group_check=True,
                    )

        def emit_B(q):
            """Pools -> T -> Lp' hi/lo fp8 stack for quarter q.

            Returns lpstk [128, 4, QC] fp8: k-tiles (Lph t0, Lph t1+pad,
            Lpl t0, Lpl t1+pad)."""
            t = yTs.tile([128, 50], f32, tag="yTs", name=f"yTsb{q}")
            nc.vector.tensor_copy(t[:], yTc[0][:, q, 0:50])
            tp = pB.tile([64, 128], f32, tag="tpTab", bufs=1, name=f"tp{q}")
            nc.tensor.transpose(tp[0:50, 0:128], t[:], idm[:])
            y_q = ypl.tile([50, QC], bf16, tag="yq", name=f"y_{q}")
            nc.scalar.activation(y_q[:], tp[0:50, 0:128], AF.Relu)
            Tab = pB.tile([128, 2, QC], f32, tag="tpTab", bufs=1, name=f"Tab{q}")
            Ta = Tab[:, 0, :]
            Tb = Tab[0:52, 1, :]
            nc.tensor.matmul(Ta, rm_sb[:, 0:128], y_q[:])
            nc.tensor.matmul(Tb, rm_sb[:, 128:180], y_q[:])
            lpstk = lpp.tile([128, 4, QC], fp8, tag="lps", name=f"lps{q}")
            lp16a = yTs.tile([128, QC], bf16, tag="lp16a", name=f"lp16a{q}")
            lp16b = yTs.tile([52, QC], bf16, tag="lp16b", name=f"lp16b{q}")
            ea = yTs.tile([128, QC], bf16, tag="ea", name=f"ea{q}")
            eb = yTs.tile([52, QC], bf16, tag="eb", name=f"eb{q}")
            # Lp' = softplus(-T) - ln2 = ln(0.5 + exp(-T - ln2))
            nc.scalar.activation(ea[:], Ta, AF.Exp, scale=-1.0,
                                 bias=nln2[:, 0:1])
            nc.scalar.activation(eb[:], Tb, AF.Exp, scale=-1.0,
                                 bias=nln2[0:52, 0:1])
            nc.scalar.activation(lp16a[:], ea[:], AF.Ln, bias=half[:, 0:1])
            nc.scalar.activation(lp16b[:], eb[:], AF.Ln, bias=half[0:52, 0:1])
            nc.vector.memset(lpstk[:, 1, :], 0.0)
            nc.vector.memset(lpstk[:, 3, :], 0.0)
            nc.scalar.activation(lpstk[:, 0, :], lp16a[:], AF.Copy)
            nc.scalar.activation(lpstk[0:52, 1, :], lp16b[:], AF.Copy)
            nc.vector.tensor_sub(lpstk[:, 2, :], lp16a[:], lpstk[:, 0, :])
            nc.vector.tensor_sub(lpstk[0:52, 3, :], lp16b[:], lpstk[0:52, 1, :])
            return lpstk

        # ---- pair-0 w1 prefetch (m-tiles 0/1), 8-chunk groups
        p0grp = [(P0G * j, min(P0G, NK - P0G * j)) for j in range((NK + P0G - 1) // P0G)]
        p0w1 = []

        def issue_p0(j):
            k0, cnt = p0grp[j]
            wtq = w1p0.tile([128, P0G, 2, 256], fp8, tag="w1q0", name=f"w1q0_{j}")
            nc.sync.dma_start(wtq[:, :cnt, :, :], w1q_d[0, :, k0 : k0 + cnt, :, :])
            p0w1.append(wtq)

        sls = (slice(0, 128), slice(128, 256))

        def consume_p0_group(k0, cnt, h, accs, started):
            """Pair-0 matmuls over chunks [k0,k0+cnt) for column half h."""
            hcols = slice(h * 2 * QC, (h + 1) * 2 * QC)
            qr = slice(2 * h, 2 * h + 2)
            for gi, lk, n, off in seam_split(k0, cnt):
                z8v = z8vs[gi]
                for jj in range(0, n, 2):
                    kk = k0 + off + jj
                    wt = p0w1[kk // P0G]
                    lw = kk - (kk // P0G) * P0G
                    rhs1 = z8v[:, lk + jj : lk + jj + 2, qr, 1, :]
                    for mi in range(2):
                        nc.tensor.matmul(
                            accs[mi][:, hcols],
                            wt[:, lw : lw + 2, 0, sls[mi]],
                            rhs1,
                            start=not started[mi],
                            stop=False,
                            perf_mode=DR,
                            skip_group_check=True,
                        )
                        started[mi] = True
                for jj in range(n):
                    kk = k0 + off + jj
                    wt = p0w1[kk // P0G]
                    lw = kk - (kk // P0G) * P0G
                    rhs2 = z8v[:, lk + jj, qr, :, :].transpose([0, 2, 1, 3])
                    last = kk == NK - 1
                    for mi in range(2):
                        nc.tensor.matmul(
                            accs[mi][:, hcols],
                            wt[:, lw, :, sls[mi]],
                            rhs2,
                            start=False,
                            stop=last,
                            perf_mode=DR,
                            skip_group_check=True,
                        )

        # ---- phase A: stream x (bf16) by halves, pools per quarter.
        # h0 DMA + pools now; p0 + h1 DMAs issued up front (h1 pools are
        # woven into C-h0 via hooks once the data has landed).
        for gi in range(len(zinfo)):
            emit_zgroup_dma(gi, 0)
            emit_zgroup_pools(gi, 0)
        for j in range(len(p0grp)):
            issue_p0(j)
        for gi in range(len(zinfo)):
            emit_zgroup_dma(gi, 1)

        selgrp = [(SELG * g, min(SELG, NK - SELG * g)) for g in range(NSG)]
        sel_tiles = {}

        def convert_group(q, sg, k0, cnt, lpstk, st):
            """Sel matmuls + gate exp + in-place fp8 packing for one
            chunk-group of one column quarter."""
            qcols = slice(q * QC, (q + 1) * QC)
            gq = psg.tile([128, SELG, QC], f32, tag="gq", bufs=2,
                          name=f"gq{q}_{sg}")
            for j in range(cnt):
                nc.tensor.matmul(
                    gq[:, j, :], st[:, j, :, :], lpstk[:, 0:2, :],
                    start=True, stop=False, perf_mode=DR,
                    skip_group_check=True,
                )
                nc.tensor.matmul(
                    gq[:, j, :], st[:, j, :, :], lpstk[:, 2:4, :],
                    start=False, stop=True, perf_mode=DR,
                    skip_group_check=True,
                )
            # gt4 = 4*G = exp(-R - ln2)
            gt4 = gtp.tile([128, SELG, QC], bf16, tag="gt", name=f"gt{q}_{sg}")
            nc.scalar.activation(gt4[:, :cnt, :], gq[:, :cnt, :], AF.Exp,
                                 scale=-1.0, bias=nln2[:, 0:1])
            stage = stp.tile([128, SELG, QC], bf16, tag="st",
                             name=f"st{q}_{sg}")
            for gi, lk, n, off in seam_split(k0, cnt):
                zt = z_tiles[gi][0]
                z8v = z8vs[gi]
                sseg = stage[:, off : off + n, :]
                nc.vector.tensor_mul(sseg, zt[:, lk : lk + n, qcols],
                                     gt4[:, off : off + n, :])
                zh = z8v[:, lk : lk + n, q, 1, :]
                zl = z8v[:, lk : lk + n, q, 0, :]
                if (2 * sg + q) % 5 < 3:
                    nc.vector.tensor_copy(zh, sseg)
                else:
                    nc.scalar.activation(zh, sseg, AF.Copy)
                sub_eng = nc.gpsimd if (2 * sg + q) % 2 == 0 else nc.vector
                sub_eng.tensor_sub(zl, sseg, zh)

        def issue_sel(sg):
            k0, cnt = selgrp[sg]
            st = sselp.tile([128, SELG, 2, 128], fp8, tag="sq",
                            name=f"sq{sg}")
            nc.sync.dma_start(st[:, :cnt, :, :], ssel_d[sg, :, :cnt, :, :])
            return st

        def emit_C(h, lpstks, hook, p0_started):
            """Gates + packing for both quarters of half h, chasing pair-0."""
            p0_backlog = []
            sts = {sg: issue_sel(sg) for sg in range(2)}
            for sg, (k0, cnt) in enumerate(selgrp):
                if sg + 2 < NSG:
                    sts[sg + 2] = issue_sel(sg + 2)
                st = sts.pop(sg)
                for q in (2 * h, 2 * h + 1):
                    convert_group(q, sg, k0, cnt, lpstks[q - 2 * h], st)
                p0_backlog.append((k0, cnt))
                if len(p0_backlog) > 2:
                    bk0, bcnt = p0_backlog.pop(0)
                    consume_p0_group(bk0, bcnt, h, accs_p0, p0_started)
                hook(h, sg)
            for bk0, bcnt in p0_backlog:
                consume_p0_group(bk0, bcnt, h, accs_p0, p0_started)

        mm2_pending = []

        w2b_sb = [None]

        def prepare_w2b(mt):
            # bf16 copy (hi+lo recombined) of the final pair's w2 slice, at
            # the fp8 scale so it accumulates into the same PSUM group
            w2t = w2p.tile([128, 2, 2, NM2, 128], fp8, tag="w2", name="w2l")
            nc.scalar.dma_start(w2t[:], w2k_d[mt // 2])
            w2b = w2bp.tile([128, 2, NM2, 128], bf16, tag="w2b", name="w2b")
            nc.vector.tensor_add(w2b[:], w2t[:, :, 0, :, :], w2t[:, :, 1, :, :])
            w2b_sb[0] = w2b

        def pack_z1(mt, accs, last=False):
            """z1 pair (m-tiles mt, mt+1) -> fp8 (zl|zh) planes. The final
            pair stays bf16 (its mm2 runs immediately; skipping the packing
            chain shortens the tail's critical path)."""
            if last:
                z1t = z1p.tile([128, 2, BS], bf16, tag="z1l", name=f"z1_{mt}")
                for mi in range(2):
                    k2 = mt + mi
                    nc.scalar.activation(z1t[:, mi, :], accs[mi][:], AF.Relu,
                                         scale=S_Z1 / S1,
                                         bias=b1_sb[:, k2 : k2 + 1])
            else:
                z1t = z1p.tile([128, 2, 2, BS], fp8, tag="z1", name=f"z1_{mt}")
                for mi in range(2):
                    k2 = mt + mi
                    stg = z1st.tile([128, BS], bf16, tag="z1s", name=f"z1s{k2}")
                    nc.scalar.activation(stg[:], accs[mi][:], AF.Relu,
                                         scale=S_Z1 / S1,
                                         bias=b1_sb[:, k2 : k2 + 1])
                    if mi == 0:
                        nc.vector.tensor_copy(z1t[:, mi, 1, :], stg[:])
                    else:
                        nc.scalar.activation(z1t[:, mi, 1, :], stg[:], AF.Copy)
                    nc.vector.tensor_sub(z1t[:, mi, 0, :], stg[:],
                                         z1t[:, mi, 1, :])
            if last:
                mm2_pending.append((mt // 2, z1t, None, last))
            else:
                w2t = w2p.tile([128, 2, 2, NM2, 128], fp8, tag="w2",
                               name=f"w2_{mt}")
                nc.scalar.dma_start(w2t[:], w2k_d[mt // 2])
                mm2_pending.append((mt // 2, z1t, w2t, last))

        def emit_mm2(m2_major=False):
            order = (
                [(e, m2) for m2 in range(NM2) for e in mm2_pending]
                if m2_major else
                [(e, m2) for e in mm2_pending for m2 in range(NM2)]
            )
            for (g2, z1t, w2t, last), m2 in order:
                if last:
                    for i in range(2):
                        nc.tensor.matmul(
                            acc2s[m2][:],
                            w2b_sb[0][:, i, m2, :],
                            z1t[:, i, :],
                            start=False,
                            stop=(g2 == NK2 // 2 - 1 and i == 1),
                            skip_group_check=True,
                        )
                    continue
                nc.tensor.matmul(
                    acc2s[m2][:],
                    w2t[:, :, 0, m2, :],
                    z1t[:, :, 1, :],
                    start=(g2 == 0),
                    stop=False,
                    perf_mode=DR,
                    skip_group_check=True,
                )
                for i in range(2):
                    nc.tensor.matmul(
                        acc2s[m2][:],
                        w2t[:, i, :, m2, :],
                        z1t[:, i, :, :],
                        start=False,
                        stop=(g2 == NK2 // 2 - 1 and i == 1),
                        perf_mode=DR,
                        skip_group_check=True,
                    )
            mm2_pending.clear()

        # ---- w1 streaming machinery (shared by C-h1 pair-1 and phase D)
        nkg = (NK + KG - 1) // KG
        kgrp = [(g * KG, min(KG, NK - g * KG)) for g in range(nkg)]
        w1_fifo = {}

        def issue_w1(mt, k0, cnt):
            wtq = w1p.tile([128, KG, 2, 256], fp8, tag="w1q")
            nc.sync.dma_start(wtq[:, :cnt, :, :],
                              w1q_d[mt // 2, :, k0 : k0 + cnt, :, :])
            w1_fifo.setdefault(mt, []).append((wtq, k0, cnt))

        def consume_w1(mt, accs):
            wtq, k0, cnt = w1_fifo[mt].pop(0)
            for gi, lk, n, off in seam_split(k0, cnt):
                z8v = z8vs[gi]
                for jj in range(0, n, 2):
                    kk = k0 + off + jj
                    rhs1 = z8v[:, lk + jj : lk + jj + 2, :, 1, :]
                    for mi in range(2):
                        nc.tensor.matmul(
                            accs[mi][:],
                            wtq[:, off + jj : off + jj + 2, 0, sls[mi]],
                            rhs1,
                            start=(kk == 0),
                            stop=False,
                            perf_mode=DR,
                            skip_group_check=True,
                        )
                for jj in range(n):
                    kk = k0 + off + jj
                    rhs2 = z8v[:, lk + jj, :, :, :].transpose([0, 2, 1, 3])
                    for mi in range(2):
                        nc.tensor.matmul(
                            accs[mi][:],
                            wtq[:, off + jj, :, sls[mi]],
                            rhs2,
                            start=False,
                            stop=(kk == NK - 1),
                            perf_mode=DR,
                            skip_group_check=True,
                        )

        def consume_w1_half(mt, accs, h, started):
            """Consume one streamed w1 group for column half h only."""
            wtq, k0, cnt = w1_fifo[mt].pop(0)
            hcols = slice(h * 2 * QC, (h + 1) * 2 * QC)
            qr = slice(2 * h, 2 * h + 2)
            for gi, lk, n, off in seam_split(k0, cnt):
                z8v = z8vs[gi]
                for jj in range(0, n, 2):
                    rhs1 = z8v[:, lk + jj : lk + jj + 2, qr, 1, :]
                    for mi in range(2):
                        nc.tensor.matmul(
                            accs[mi][:, hcols],
                            wtq[:, off + jj : off + jj + 2, 0, sls[mi]],
                            rhs1,
                            start=not started[mi],
                            stop=False,
                            perf_mode=DR,
                            skip_group_check=True,
                        )
                        started[mi] = True
                for jj in range(n):
                    kk = k0 + off + jj
                    rhs2 = z8v[:, lk + jj, qr, :, :].transpose([0, 2, 1, 3])
                    for mi in range(2):
                        nc.tensor.matmul(
                            accs[mi][:, hcols],
                            wtq[:, off + jj, :, sls[mi]],
                            rhs2,
                            start=False,
                            stop=(kk == NK - 1),
                            perf_mode=DR,
                            skip_group_check=True,
                        )

        # ---- phases B/C. C-h0 carries: pools-h1, pair-A's h0 columns.
        # C-h1 carries: pair-A's h1 columns (w1 re-streamed) + pair-B full.
        h1_state = [0]
        pA_state = [0]
        pA_started = [False, False]

        def advance_pA(h):
            g = pA_state[0]
            if g + 1 < nkg:
                issue_w1(2, *kgrp[g + 1])
            consume_w1_half(2, accs_pA, h, pA_started)
            pA_state[0] += 1

        def hook_h0(h, sg):
            if sg >= 2:
                frontier = SELG * (sg - 2) + selgrp[sg - 2][1]
                while pA_state[0] < nkg:
                    g = pA_state[0]
                    # pace to the w1 DMA stream (it contends with xt-h1 and
                    # the pair-0 prefetch): don't enqueue PE work whose
                    # weights can't have landed yet
                    if sg < 8 + (g * 3) // 2:
                        break
                    if kgrp[g][0] + kgrp[g][1] > frontier:
                        break
                    advance_pA(0)
            if sg >= 14 and h1_state[0] < len(zinfo):
                for _ in range(3):
                    if h1_state[0] < len(zinfo):
                        emit_zgroup_pools(h1_state[0], 1)
                        h1_state[0] += 1

        lps0 = emit_B(0)
        lps1 = emit_B(1)
        issue_w1(2, *kgrp[0])
        emit_C(0, (lps0, lps1), hook_h0, [False, False])
        pA_h0_left = list(range(pA_state[0], nkg))
        while h1_state[0] < len(zinfo):
            emit_zgroup_pools(h1_state[0], 1)
            h1_state[0] += 1
        lps2 = emit_B(2)
        lps3 = emit_B(3)

        bstack.close()
        astack = ExitStack()
        psmD = astack.enter_context(tc.tile_pool(name="psmD", bufs=1, space="PSUM"))
        accs_pB = (
            psmD.tile([128, BS], f32, tag="pBa", name="acc0pB"),
            psmD.tile([128, BS], f32, tag="pBb", name="acc1pB"),
        )
        pA_h0_started = pA_started

        def advance_pA_h0flush():
            g = pA_state[0]
            if g + 1 < nkg:
                issue_w1(2, *kgrp[g + 1])
            consume_w1_half(2, accs_pA, 0, pA_h0_started)
            pA_state[0] += 1

        pA1_state = [0]
        pA_started = [False, False]
        pB_state = [0]
        pB_started = [False, False]
        issue_w1(4, *kgrp[0])

        pA1_fifo_seeded = [False]

        def advance_pA1():
            if not pA1_fifo_seeded[0]:
                issue_w1(2, *kgrp[0])
                pA1_fifo_seeded[0] = True
            g = pA1_state[0]
            if g + 1 < nkg:
                issue_w1(2, *kgrp[g + 1])
            consume_w1_half(2, accs_pA, 1, pA_started)
            pA1_state[0] += 1

        def advance_pB():
            g = pB_state[0]
            if g + 1 < nkg:
                issue_w1(4, *kgrp[g + 1])
            consume_w1(4, accs_pB)
            pB_state[0] += 1

        def hook_h1(h, sg):
            if sg < 2:
                return
            frontier = SELG * (sg - 2) + selgrp[sg - 2][1]
            while True:
                if pA_h0_left:
                    pA_h0_left.pop(0)
                    advance_pA_h0flush()
                    continue
                ja = pA1_state[0]
                jb = pB_state[0]
                if ja < nkg and (jb >= nkg or ja <= jb):
                    if kgrp[ja][0] + kgrp[ja][1] > frontier:
                        break
                    advance_pA1()
                elif jb < nkg:
                    if kgrp[jb][0] + kgrp[jb][1] > frontier:
                        break
                    advance_pB()
                else:
                    break

        emit_C(1, (lps2, lps3), hook_h1, [False, False])
        while pA_h0_left:
            pA_h0_left.pop(0)
            advance_pA_h0flush()
        while pA1_state[0] < nkg:
            advance_pA1()
        while pB_state[0] < nkg:
            advance_pB()

        # ---- evict pairs 0, A, B; start mm2 accumulators
        pack_z1(0, accs_p0)
        pack_z1(2, accs_pA)
        pack_z1(4, accs_pB)
        astack.close()
        psmI.close()
        cstack.close()

        dstack = ExitStack()
        psm = dstack.enter_context(tc.tile_pool(name="psm", bufs=1, space="PSUM"))
        w2bp = dstack.enter_context(tc.tile_pool(name="w2bp", bufs=1))
        e2stack = ExitStack()
        psm2 = e2stack.enter_context(tc.tile_pool(name="psm2", bufs=1, space="PSUM"))
        acc2s = [
            psm2.tile([128, BS], f32, tag=f"mm2_{m2}", name=f"acc2_{m2}")
            for m2 in range(NM2)
        ]

        # ---- phase D: pairs 3..15 full-width, mm2 consumed mid-pair
        issue_w1(6, *kgrp[0])
        for mt in range(6, NM1, 2):
            if mt == NM1 - 2:
                prepare_w2b(mt)
            acc0 = psm.tile([128, BS], f32, tag="mm1a", bufs=2, name=f"acc0_{mt}")
            acc1 = psm.tile([128, BS], f32, tag="mm1b", bufs=2, name=f"acc1_{mt}")
            for g in range(nkg):
                if g + 1 < nkg:
                    issue_w1(mt, *kgrp[g + 1])
                elif mt + 2 < NM1:
                    issue_w1(mt + 2, *kgrp[0])
                consume_w1(mt, (acc0, acc1))
                if g == nkg // 2:
                    emit_mm2()
            pack_z1(mt, (acc0, acc1), last=(mt == NM1 - 2))
        emit_mm2(m2_major=True)

        # ---- phase E: z2 = relu(acc2/S2 + b2) feeding fused mm3
        with (
            tc.tile_pool(name="z2p", bufs=2) as z2p,
            tc.tile_pool(name="tailp", bufs=1) as tailp,
        ):
            acc3 = psm.tile([3, BS], f32, tag="mm1a", bufs=2, name="acc3")
            for m2 in range(NM2):
                z2t = z2p.tile([128, BS], f32r, tag="z2")
                nc.scalar.activation(
                    z2t[:], acc2s[m2][:], AF.Relu, scale=1.0 / S2,
                    bias=b2_sb[:, m2 : m2 + 1]
                )
                nc.tensor.matmul(
                    acc3[:], w3_sb[:, m2, :], z2t[:],
                    start=(m2 == 0), stop=False,
                    skip_group_check=True,
                )
            nc.tensor.matmul(
                acc3[:], b3_sb[:], ones1B[:],
                start=False, stop=True,
                skip_group_check=True,
            )
            # ---- phase F: out = z3 / ||z3||, with 1/sqrt(s) computed as
            # exp(-0.5 ln s) + one Newton step (avoids the Sqrt act-table
            # load; exp/ln/square live in the already-loaded table)
            sq = tailp.tile([3, BS], f32r)
            nc.scalar.activation(sq[:], acc3[:], AF.Square)
            z3sb = tailp.tile([3, BS], f32)
            nc.scalar.copy(z3sb[:], acc3[:])
            sps = psm2.tile([128, BS], f32, tag="mm2_0", name="sps")
            nc.tensor.matmul(sps[0:1, :], ones31[:], sq[:])
            rts = tailp.tile([1, BS], f32)
            nc.scalar.activation(rts[:], sps[0:1, :], AF.Sqrt)
            inv = tailp.tile([1, BS], f32r)
            with nc.allow_low_precision(reason="f32r is fp32-width storage"):
                nc.vector.reciprocal(inv[:], rts[:])
            inv3 = psm2.tile([128, BS], f32, tag="mm2_1", name="inv3")
            nc.tensor.matmul(inv3[0:3, :], ones13[:], inv[:])
            outt = tailp.tile([3, BS], f32)
            nc.vector.tensor_mul(outt[:], z3sb[:], inv3[0:3, :])
            nc.sync.dma_start(out_d[:], outt[:])
        e2stack.close()
        dstack.close()

    nc.compile()
    return nc


def _prep_shared(conv_w, F_w, w1, b1, w2, b2, w3, b3):
    """Host-side weight layouts shared by all cores."""
    fa = np.arange(F)
    c_idx = fa // 4000
    d_idx = (fa // 400) % 10
    h_idx = (fa // 40) % 10
    w_idx = fa % 40

    # pooled conv: y_pre[j] = sum_f wp[f, j] * x^T[f, b]
    wp = np.zeros((NWPG * WPG * 128, 50), np.float32)
    wp[fa, h_idx] = conv_w[c_idx] / 400.0
    wp[fa, 10 + w_idx] = conv_w[c_idx] / 100.0
    wpool = np.ascontiguousarray(
        wp.reshape(NWPG, WPG, 128, 50).transpose(0, 2, 1, 3)
    ).astype(np_bf16)

    # rmat: T[r, b] = sum_j rmat[j, r] * y[j, b]
    rm = np.zeros((50, 180), np.float32)
    cc10 = np.repeat(np.arange(3), 10)
    rm[np.tile(np.arange(10), 3), np.arange(30)] = F_w[cc10]
    rm[np.tile(np.arange(10), 3), 30 + np.arange(30)] = F_w[cc10]
    cc40 = np.repeat(np.arange(3), 40)
    rm[10 + np.tile(np.arange(40), 3), 60 + np.arange(120)] = F_w[cc40]

    # selection: logG[f] = sel[f, :] @ Lp'  (exact 0/1 weights, fp8)
    sel = np.zeros((FP, 180), np.float32)
    sel[fa, c_idx * 10 + d_idx] = 1.0
    sel[fa, 30 + c_idx * 10 + h_idx] = 1.0
    sel[fa, 60 + c_idx * 40 + w_idx] = 1.0
    sel_full = np.zeros((NSG * SELG * 128, 256), np.float32)
    sel_full[:FP, :180] = sel
    ssel = np.ascontiguousarray(
        sel_full.reshape(NSG, SELG, 128, 2, 128).transpose(0, 4, 1, 3, 2)
    ).astype(np_fp8)

    # w1: fp8 hi/lo pair, scaled by S_W1
    w1p_ = np.zeros((H1, FP), np.float32)
    w1p_[:, :F] = w1
    wt = w1p_.T * S_W1  # [FP, H1]
    wh8 = wt.astype(np_fp8)
    wl8 = (wt - wh8.astype(np.float32)).astype(np_fp8)
    w1q = np.zeros((NM1 // 2, 128, NK, 2, 256), np_fp8)
    w1q[:, :, :, 0, :] = wh8.reshape(NK, 128, NM1 // 2, 256).transpose(2, 1, 0, 3)
    w1q[:, :, :, 1, :] = wl8.reshape(NK, 128, NM1 // 2, 256).transpose(2, 1, 0, 3)

    # w2: fp8 hi/lo pair, scaled by S_W2
    w2t_ = w2.T * S_W2  # [H1, H2]
    wh2 = w2t_.astype(np_fp8)
    wl2 = (w2t_ - wh2.astype(np.float32)).astype(np_fp8)
    w2k = np.zeros((NK2 // 2, 128, 2, 2, NM2, 128), np_fp8)
    w2k[:, :, :, 0, :, :] = wh2.reshape(NK2 // 2, 2, 128, NM2, 128).transpose(
        0, 2, 1, 3, 4
    )
    w2k[:, :, :, 1, :, :] = wl2.reshape(NK2 // 2, 2, 128, NM2, 128).transpose(
        0, 2, 1, 3, 4
    )

    w3h = np.ascontiguousarray(w3.reshape(3, NM2, 128).transpose(2, 1, 0))

    return {
        "wpool": wpool,
        "rmat": rm.astype(np_bf16),
        "ssel": ssel,
        "w1q": w1q,
        "w2k": w2k,
        "w3h": w3h,
        "b1g4": np.ascontiguousarray((S_Z1 * b1).reshape(NM1, 128).T),
        "b2g": np.ascontiguousarray(b2.reshape(NM2, 128).T),
        "b3r": np.ascontiguousarray(b3.reshape(1, 3)),
        "idm": np.eye(128, dtype=np.float32),
    }


def make_in_maps(x, conv_w, F_w, w1, b1, w2, b2, w3, b3):
    x = np.asarray(x, np.float32).reshape(B_TOT, F)
    shared = _prep_shared(
        np.asarray(conv_w, np.float32).reshape(3),
        np.asarray(F_w, np.float32).reshape(3),
        np.asarray(w1, np.float32),
        np.asarray(b1, np.float32),
        np.asarray(w2, np.float32),
        np.asarray(b2, np.float32),
        np.asarray(w3, np.float32),
        np.asarray(b3, np.float32),
    )
    in_maps = []
    for c in range(N_CORES):
        xs = x[c * BS : (c + 1) * BS]
        xt = np.zeros((FP, BS), np_bf16)
        xt[:F] = xs.T.astype(np_bf16)
        m = dict(shared)
        m["xt"] = np.ascontiguousarray(
            xt.reshape(NK, 128, BS).transpose(1, 0, 2)
        )
        in_maps.append(m)
    return in_maps


def get_nc():
    if "nc" not in _NC_CACHE:
        _NC_CACHE["nc"] = build_nc()
    return _NC_CACHE["nc"]


def kernel(**inputs) -> np.ndarray:
    nc = get_nc()
    in_maps = make_in_maps(**inputs)
    res = run_bass_kernel_spmd(nc, in_maps, core_ids=list(range(N_CORES)))
    out = np.concatenate([r["out"] for r in res.results], axis=1)  # [3, 4096]
    return np.ascontiguousarray(out.T, dtype=np.float32)
